# revision 22
# baseline (speedup 1.0000x reference)
"""Trainium2 Bass kernel for nn_ChannelSelfAttention.

Reference computation (per batch sample b):
    xt   = x[b].T                          # [C, L]
    q    = xt @ Wq.T + bq                  # [C, H]
    kv   = xt @ Wkv.T + bkv                # [C, 2H] -> k, v
    attn = (q * H**-0.5) @ k.T             # [C, C]  (no softmax)
    y    = attn @ v                        # [C, H]
    g    = mean(y, axis=-1)                # [C]
    out[b] = x[b] * g[None, :]             # [L, C]

Sharding: data-parallel over B across 8 cores (4 samples per core);
weights replicated.

This problem is HBM-bandwidth bound, so all DRAM I/O is fp16: the host
casts x/W to fp16 (the attention scale 1/sqrt(64) = 1/8 is folded into
Wq exactly) and the kernel writes fp16 output which the host widens
back to f32. That halves DMA bytes vs f32 (36.7 MB -> 18.4 MB per
core). The contraction over L=4096 accumulates in f32 PSUM and the
whole [C,C] attention stage runs in f32, so the only precision loss is
the fp16 rounding of x/W/out (~1e-3 rel).

On-device layout (per sample): l = p*32 + m, i.e. x[b] is just
reshaped [128, 32, 256] — partition-contiguous 16 KB DMA descriptors,
no host transpose needed for either input or output.

  - qkv^T accumulated over the 32 m-chunks (lhsT = fp16 W^T chunk,
    rhs = fp16 x chunk) -> q^T [64, 256], kv^T [128, 256] in f32 PSUM.
  - bias adds on the Act engine (per-partition bias AP, scale folded
    into the weights host-side), keeping DVE free for the gate.
  - attn^T[d, c] = k^T-chunk.T @ q^T directly (f32r), avoiding an
    attn transpose; v^T is PE-transposed to v for the y^T matmul.
  - g = mean_h y^T via a (1/H)-ones matmul that also broadcasts g to
    all 128 partitions; Act converts it to fp16.
  - gate on DVE in pure fp16 (2x_1p fast path): out = x * g with g
    broadcast along m via a stride-0 access pattern.
"""

import numpy as np

import concourse.bass as bass
import concourse.mybir as mybir
import concourse.tile as tile
from concourse import bacc
from concourse.bass_utils import run_bass_kernel_spmd

B, L, C, H = 32, 4096, 256, 64
N_CORES = 8
B_LOC = B // N_CORES          # samples per core
P = 128                       # SBUF partitions
M = L // P                    # l-rows per partition (l = p*M + m)
DCH = C // P                  # d-chunks (2)
F16 = mybir.dt.float16
F32 = mybir.dt.float32
F32R = mybir.dt.float32r
SCALE = float(H) ** -0.5      # exactly 1/8; folded into Wq on host


def _emit(tc: "tile.TileContext", x_d, wT_d, bq_d, bkv_d, id_d, ones_d, out_d):
    nc = tc.nc
    QT = M // 4                                      # x load quarter (8 chunks)
    HF = M // 2                                      # gate/store half (16 chunks)
    with (
        tc.tile_pool(name="singles", bufs=1) as singles,
        tc.tile_pool(name="xin", bufs=4) as xin,
        tc.tile_pool(name="xout", bufs=3) as xout,
        tc.tile_pool(name="small", bufs=2) as small,
        tc.tile_pool(name="psum2", bufs=2, space="PSUM") as psum2,
        tc.tile_pool(name="psum1", bufs=1, space="PSUM") as psum1,
    ):
        # ---- one-time loads (scalar HWDGE ring; overlaps x loads on the
        # sync ring). First weight chunk loaded separately so sample 0's
        # first matmuls don't gate on the full 1.5 MiB. ----
        wT_sb = singles.tile([P, M, 3 * H], F16)     # 1.5 MiB
        nc.scalar.dma_start(out=wT_sb[:, 0:1], in_=wT_d[:, 0:1])
        nc.scalar.dma_start(out=wT_sb[:, 1:M], in_=wT_d[:, 1:M])
        bq_sb = singles.tile([H, 1], F32)            # already scaled by 1/8
        nc.scalar.dma_start(out=bq_sb, in_=bq_d[:].rearrange("(h o) -> h o", o=1))
        bkv_sb = singles.tile([2 * H, 1], F32)
        nc.scalar.dma_start(out=bkv_sb, in_=bkv_d[:].rearrange("(h o) -> h o", o=1))
        # 64x64 identity at partitions 64:128 so the v^T transpose
        # (lhsT at base partition 64) has a base-aligned rhs.
        id_tile = singles.tile([P, H], F32R)
        nc.scalar.dma_start(out=id_tile[H:P, :], in_=id_d[:])
        ident_hi = id_tile[H:P, :]
        ones_h = singles.tile([H, P], F32R)          # filled with 1/H
        nc.scalar.dma_start(out=ones_h, in_=ones_d[:])

        for b in range(B_LOC):
            # ---- load x[b] into SBUF as [128, 32*256] fp16, in quarters
            # (4 KB per-partition descriptors) so qkv can start early ----
            x_sb = xin.tile([P, M * C], F16, tag="x")
            for qt in range(4):
                sl = slice(qt * QT * C, (qt + 1) * QT * C)
                nc.sync.dma_start(out=x_sb[:, sl], in_=x_d[b][:, sl])

            # ---- qkv^T = W_all @ x[b]: accumulate over 32 m-chunks ----
            psum_q = psum2.tile([H, C], F32, tag="q")
            for m in range(M):
                nc.tensor.matmul(
                    psum_q,
                    lhsT=wT_sb[:, m, 0:H],
                    rhs=x_sb[:, m * C : (m + 1) * C],
                    start=(m == 0),
                    stop=(m == M - 1),
                )
            psum_kv = psum2.tile([2 * H, C], F32, tag="kv")
            for m in range(M):
                nc.tensor.matmul(
                    psum_kv,
                    lhsT=wT_sb[:, m, H : 3 * H],
                    rhs=x_sb[:, m * C : (m + 1) * C],
                    start=(m == 0),
                    stop=(m == M - 1),
                )

            # bias adds on Act (scale already folded into Wq/bq host-side)
            q_sb = small.tile([H, C], F32R, tag="q_sb")
            nc.scalar.activation(
                q_sb[:],
                psum_q,
                mybir.ActivationFunctionType.Identity,
                bias=bq_sb[:],
            )
            kv_sb = small.tile([2 * H, C], F32R, tag="kv_sb")
            nc.scalar.activation(
                kv_sb[:],
                psum_kv,
                mybir.ActivationFunctionType.Identity,
                bias=bkv_sb[:],
            )
            kT = kv_sb[0:H, :]                    # [64, 256]
            vT = kv_sb[H : 2 * H, :]              # [64, 256]

            # ---- v natural [d, h]: PE-transpose the two vT halves ----
            psum_vt = psum1.tile([P, P], F32R, tag="vt")
            for d in range(DCH):
                nc.tensor.transpose(
                    psum_vt[:, d * H : (d + 1) * H],
                    vT[:, d * P : (d + 1) * P],
                    ident_hi,
                )
            v_sb = small.tile([P, P], F32R, tag="v_sb")
            nc.scalar.copy(v_sb, psum_vt)

            # ---- attn^T[d, c] = sum_h k^T[h, d] * q^T[h, c] ----
            psum_at = psum1.tile([P, DCH * C], F32, tag="at")
            for d in range(DCH):
                nc.tensor.matmul(
                    psum_at[:, d * C : (d + 1) * C],
                    lhsT=kT[:, d * P : (d + 1) * P],
                    rhs=q_sb[:],
                )
            at_sb = small.tile([P, DCH * C], F32R, tag="at_sb")
            nc.scalar.copy(at_sb, psum_at)

            # ---- y^T[h, c] = sum_d v[d, h] * attn^T[d, c] ----
            psum_yt = psum1.tile([H, C], F32, tag="yt")
            for d in range(DCH):
                nc.tensor.matmul(
                    psum_yt,
                    lhsT=v_sb[:, d * H : (d + 1) * H],
                    rhs=at_sb[:, d * C : (d + 1) * C],
                    start=(d == 0),
                    stop=(d == DCH - 1),
                )
            yt_sb = small.tile([H, C], F32R, tag="yt_sb")
            nc.scalar.copy(yt_sb, psum_yt)

            # ---- g = mean_h y^T broadcast to all 128 partitions, fp16 ----
            psum_g = psum1.tile([P, C], F32, tag="g")
            nc.tensor.matmul(psum_g, lhsT=ones_h[:], rhs=yt_sb[:])
            g16 = small.tile([P, C], F16, tag="g16")
            nc.scalar.copy(g16, psum_g)

            # ---- gate: out = x * g, all fp16 on DVE (2x_1p), g broadcast
            # along m via stride 0; quarters so stores drain smoothly ----
            out_sb = xout.tile([P, M * C], F16, tag="out")
            g_bc = bass.AP(
                tensor=g16.tensor,
                offset=g16.offset,
                ap=[list(g16.ap[0]), [0, QT], list(g16.ap[1])],
            )
            for hh in range(4):
                sl = slice(hh * QT * C, (hh + 1) * QT * C)
                nc.vector.tensor_tensor(
                    out=out_sb[:, sl].rearrange("p (m c) -> p m c", c=C),
                    in0=x_sb[:, sl].rearrange("p (m c) -> p m c", c=C),
                    in1=g_bc,
                    op=mybir.AluOpType.mult,
                )
                nc.scalar.dma_start(out=out_d[b][:, sl], in_=out_sb[:, sl])


def build():
    nc = bacc.Bacc(
        "TRN2", target_bir_lowering=False, debug=False, num_devices=N_CORES
    )
    x_d = nc.dram_tensor("x", [B_LOC, P, M * C], F16, kind="ExternalInput")
    wT_d = nc.dram_tensor("wT", [P, M, 3 * H], F16, kind="ExternalInput")
    bq_d = nc.dram_tensor("bq", [H], F32, kind="ExternalInput")
    bkv_d = nc.dram_tensor("bkv", [2 * H], F32, kind="ExternalInput")
    id_d = nc.dram_tensor("ident", [H, H], F32R, kind="ExternalInput")
    ones_d = nc.dram_tensor("ones", [H, P], F32R, kind="ExternalInput")
    out_d = nc.dram_tensor("out", [B_LOC, P, M * C], F16, kind="ExternalOutput")
    with tile.TileContext(nc) as tc:
        _emit(tc, x_d, wT_d, bq_d, bkv_d, id_d, ones_d, out_d)
    nc.compile()
    return nc


_nc_cache = None


def _get_nc():
    global _nc_cache
    if _nc_cache is None:
        _nc_cache = build()
    return _nc_cache


def make_in_maps(x, Wq, bq, Wkv, bkv):
    x16 = np.asarray(x, dtype=np.float32).astype(np.float16)
    wT = (
        np.concatenate(
            [np.asarray(Wq, np.float32) * SCALE, np.asarray(Wkv, np.float32)],
            axis=0,
        )
        .T.astype(np.float16)
        .reshape(P, M, 3 * H)
    )
    wT = np.ascontiguousarray(wT)
    bq_s = np.ascontiguousarray(np.asarray(bq, np.float32) * SCALE)
    bkv_c = np.ascontiguousarray(np.asarray(bkv, np.float32))
    ident = np.eye(H, dtype=np.float32)
    ones = np.full((H, P), 1.0 / H, dtype=np.float32)
    return [
        {
            "x": x16[i * B_LOC : (i + 1) * B_LOC].reshape(B_LOC, P, M * C),
            "wT": wT,
            "bq": bq_s,
            "bkv": bkv_c,
            "ident": ident,
            "ones": ones,
        }
        for i in range(N_CORES)
    ]


def run(inputs, **spmd_kwargs):
    """Run on hardware; returns (full_output, BassKernelResults)."""
    nc = _get_nc()
    in_maps = make_in_maps(**inputs)
    res = run_bass_kernel_spmd(nc, in_maps, list(range(N_CORES)), **spmd_kwargs)
    out = np.concatenate(
        [np.asarray(r["out"]).reshape(B_LOC, L, C) for r in res.results], axis=0
    ).astype(np.float32)
    return out, res


def kernel(**inputs) -> np.ndarray:
    out, _ = run(inputs)
    return out
